# revision 8
# baseline (speedup 1.0000x reference)
"""Trainium2 Bass kernel for nn_AttendModule (sparse spatial attention).

Computation (per batch b):
    Q = (Wq/sqrt(C2)) @ x ; M = Wm @ x ; V = Wv @ x          (1x1x1 conv projections)
    A[q, t, s] = <Q[:, q], M[:, t*S+s]>                       (QK over channels)
    Att = softmax over s (spatial positions within each key frame t)
    R[c, t, q] = sum_s Att[q, t, s] * V[c, t*S+s]
    returns (R reshaped (B, C2, T, T, H, W), V (B, C2, T, H, W))

Sharding over 8 NeuronCores: core = (b, th) with b = core//2 batch and
th = core%2 a half of the key-frame axis (8 key frames each). Queries are
kept whole on every core; key frames are disjoint, so no collectives.

Per-core dataflow (all on one core, SPMD program identical across cores):
    - projections via TensorE (bf16 operands, fp32 PSUM accumulation;
      V projection in full fp32 since V is returned directly)
    - A^T tiles (keys x queries) = M_sub^T @ Q via TensorE fp32-accum
    - exp on ScalarE (PSUM -> bf16 SBUF)
    - fused AV pass: for each 128-query subtile, R[q, t, c] accumulates
      E_j^T @ V^T_j (N=128) and, reusing the same stationary operand,
      D[q, t] accumulates E_j^T @ sel_j (N=8) -> softmax denominators
      land in query-partition layout, so the final normalize is a single
      DVE tensor_mul with a broadcast access pattern fused with the
      PSUM->SBUF evacuation.
"""
import os
import sys

sys.path.insert(0, "/opt/trn_rl_repo")

import numpy as np
import ml_dtypes

import concourse.bass as bass
import concourse.bacc as bacc
import concourse.mybir as mybir
import concourse.tile as tile
from concourse import bass_utils

B, C, T, H, W = 4, 256, 16, 16, 16
C2 = 128
S = H * W            # 256 spatial positions per frame
THW = T * S          # 4096 flattened query positions
TK = T // 2          # 8 key frames per core
SK = TK * S          # 2048 key positions per core
NCH = THW // 512     # 8 query chunks of 512

F32 = mybir.dt.float32
F32R = mybir.dt.float32r
BF16 = mybir.dt.bfloat16
EXP = mybir.ActivationFunctionType.Exp

LAST_EXEC_TIME_NS = None
LAST_RESULTS = None

_cache = {}


def _build_program():
    nc = bacc.Bacc("TRN2", target_bir_lowering=False, debug=False)

    xqb_d = nc.dram_tensor("xqb", [C, THW], BF16, kind="ExternalInput")
    xkv_d = nc.dram_tensor("xkv", [C, SK], F32, kind="ExternalInput")
    xkvb_d = nc.dram_tensor("xkvb", [C, SK], BF16, kind="ExternalInput")
    wqtb_d = nc.dram_tensor("wqtb", [C, C2], BF16, kind="ExternalInput")
    wmtb_d = nc.dram_tensor("wmtb", [C, C2], BF16, kind="ExternalInput")
    wvt_d = nc.dram_tensor("wvt", [C, C2], F32, kind="ExternalInput")
    wvtb_d = nc.dram_tensor("wvtb", [C, C2], BF16, kind="ExternalInput")
    sel_d = nc.dram_tensor("sel", [128, 128], BF16, kind="ExternalInput")
    r_d = nc.dram_tensor("r", [THW, TK, C2], F32, kind="ExternalOutput")
    v_d = nc.dram_tensor("v", [C2, SK], F32, kind="ExternalOutput")

    with tile.TileContext(nc) as tc:
        with (
            tc.tile_pool(name="const", bufs=1) as cpool,
            tc.tile_pool(name="xp", bufs=1) as xpool,
            tc.tile_pool(name="big", bufs=1) as bpool,
            tc.tile_pool(name="ep", bufs=32) as epool,
            tc.tile_pool(name="rp", bufs=4) as rpool,
            tc.tile_pool(name="pa", bufs=3, space="PSUM") as pa_pool,
            tc.tile_pool(name="pr", bufs=2, space="PSUM") as pr_pool,
            tc.tile_pool(name="pd", bufs=1, space="PSUM") as pd_pool,
        ):
            # ---- load inputs to SBUF
            xqb = [xpool.tile([128, THW], BF16, tag=f"xqb{h}", name=f"xqb{h}") for h in range(2)]
            xkv = [xpool.tile([128, SK], F32, tag=f"xkv{h}", name=f"xkv{h}") for h in range(2)]
            xkvb = [xpool.tile([128, SK], BF16, tag=f"xkvb{h}", name=f"xkvb{h}") for h in range(2)]
            wq = [cpool.tile([128, C2], BF16, tag=f"wq{h}", name=f"wq{h}") for h in range(2)]
            wm = [cpool.tile([128, C2], BF16, tag=f"wm{h}", name=f"wm{h}") for h in range(2)]
            wv = [cpool.tile([128, C2], F32, tag=f"wv{h}", name=f"wv{h}") for h in range(2)]
            wvb = [cpool.tile([128, C2], BF16, tag=f"wvb{h}", name=f"wvb{h}") for h in range(2)]
            for h in range(2):
                hs = slice(h * 128, (h + 1) * 128)
                nc.sync.dma_start(wq[h][:], wqtb_d.ap()[hs, :])
                nc.sync.dma_start(wm[h][:], wmtb_d.ap()[hs, :])
                nc.sync.dma_start(wv[h][:], wvt_d.ap()[hs, :])
                nc.sync.dma_start(wvb[h][:], wvtb_d.ap()[hs, :])
            sel = cpool.tile([128, 128], BF16, tag="sel")
            nc.sync.dma_start(sel[:], sel_d.ap())
            for c in range(0, THW, 1024):
                cs = slice(c, c + 1024)
                for h in range(2):
                    hs = slice(h * 128, (h + 1) * 128)
                    nc.gpsimd.dma_start(xqb[h][:, cs], xqb_d.ap()[hs, cs])
                    if c < SK:
                        nc.gpsimd.dma_start(xkvb[h][:, cs], xkvb_d.ap()[hs, cs])
                        nc.gpsimd.dma_start(xkv[h][:, cs], xkv_d.ap()[hs, cs])

            Q_sb = bpool.tile([128, THW], BF16, tag="Q")
            M_sb = bpool.tile([128, SK], BF16, tag="M")
            V_sb = bpool.tile([128, SK], F32, tag="V")
            VT_sb = bpool.tile([128, SK], BF16, tag="VT")

            # ---- projections
            for n in range(NCH):
                ns = slice(n * 512, (n + 1) * 512)
                p = pa_pool.tile([128, 512], F32, tag="pa")
                nc.tensor.matmul(p[:], wq[0][:], xqb[0][:, ns], start=True, stop=False)
                nc.tensor.matmul(p[:], wq[1][:], xqb[1][:, ns], start=False, stop=True)
                nc.scalar.copy(Q_sb[:, ns], p[:])
            for n in range(SK // 512):
                ns = slice(n * 512, (n + 1) * 512)
                p = pa_pool.tile([128, 512], F32, tag="pa")
                nc.tensor.matmul(p[:], wm[0][:], xkvb[0][:, ns], start=True, stop=False)
                nc.tensor.matmul(p[:], wm[1][:], xkvb[1][:, ns], start=False, stop=True)
                nc.scalar.copy(M_sb[:, ns], p[:])
            # V^T (key-partition layout) for the AV matmul, bf16
            for j in range(16):
                js = slice(j * 128, (j + 1) * 128)
                p = pd_pool.tile([128, 128], F32, tag="pd")
                nc.tensor.matmul(p[:], xkvb[0][:, js], wvb[0][:], start=True, stop=False)
                nc.tensor.matmul(p[:], xkvb[1][:, js], wvb[1][:], start=False, stop=True)
                nc.scalar.copy(VT_sb[:, js], p[:])
            for n in range(SK // 512):
                ns = slice(n * 512, (n + 1) * 512)
                p = pa_pool.tile([128, 512], F32, tag="pa")
                nc.tensor.matmul(p[:], wv[0][:], xkv[0][:, ns], start=True, stop=False)
                nc.tensor.matmul(p[:], wv[1][:], xkv[1][:, ns], start=False, stop=True)
                nc.scalar.copy(V_sb[:, ns], p[:])
            nc.gpsimd.dma_start(v_d.ap(), V_sb[:])

            # ---- attention, one 512-query chunk at a time
            for n in range(NCH):
                ns = slice(n * 512, (n + 1) * 512)
                E = []
                for j in range(16):
                    js = slice(j * 128, (j + 1) * 128)
                    p = pa_pool.tile([128, 512], F32, tag="pa")
                    nc.tensor.matmul(p[:], M_sb[:, js], Q_sb[:, ns], start=True, stop=True)
                    e = epool.tile([128, 512], BF16, tag="E")
                    nc.scalar.activation(e[:], p[:], EXP)
                    E.append(e)
                rsb = rpool.tile([128, 4, TK, C2], F32, tag="rsb")
                for u in range(4):
                    us = slice(u * 128, (u + 1) * 128)
                    pr = pr_pool.tile([128, TK, C2], F32, tag="pr")
                    pdt = pd_pool.tile([128, TK], F32, tag="pd")
                    for j in range(16):
                        t = j // 2
                        nc.tensor.matmul(
                            pr[:, t, :], E[j][:, us], VT_sb[:, j * 128:(j + 1) * 128],
                            start=(j % 8 == 0), stop=(j % 8 == 7),
                        )
                        nc.tensor.matmul(
                            pdt[:], E[j][:, us], sel[:, j * 8:(j + 1) * 8],
                            start=(j == 0), stop=(j == 15),
                        )
                    rdt = rpool.tile([128, TK], F32, tag="rdt")
                    if os.environ.get("ATT_SAFE_RECIP", "0") == "1":
                        nc.vector.reciprocal(out=rdt[:], in_=pdt[:])
                    else:
                        nc.vector.reciprocal_approx_fast(out=rdt[:], in_=pdt[:])
                    for t in range(TK):
                        nc.vector.tensor_scalar_mul(
                            rsb[:, u, t, :], pr[:, t, :], rdt[:, t:t + 1]
                        )
                dst = r_d.ap()[n * 512:(n + 1) * 512, :, :].rearrange(
                    "(u p) t c -> p u t c", p=128)
                nc.sync.dma_start(dst, rsb[:])

    nc.compile()
    return nc


def _get_program():
    if "nc" not in _cache:
        _cache["nc"] = _build_program()
    return _cache["nc"]


def _prep_in_maps(x, Wq, Wm, Wv):
    x = np.asarray(x, np.float32)
    Wq = np.asarray(Wq, np.float32)
    Wm = np.asarray(Wm, np.float32)
    Wv = np.asarray(Wv, np.float32)

    inv = np.float32(1.0 / np.sqrt(np.float32(C2)))
    xf = np.ascontiguousarray(x.reshape(B, C, THW))
    xfb = xf.astype(ml_dtypes.bfloat16)
    wqtb = np.ascontiguousarray((Wq.T * inv).astype(ml_dtypes.bfloat16))
    wmtb = np.ascontiguousarray(Wm.T.astype(ml_dtypes.bfloat16))
    wvt = np.ascontiguousarray(Wv.T)
    wvtb = wvt.astype(ml_dtypes.bfloat16)
    sel = np.zeros((128, 128), ml_dtypes.bfloat16)
    for j in range(16):
        sel[:, 8 * j + j // 2] = 1.0

    in_maps = []
    for core in range(8):
        b, th = core // 2, core % 2
        ksl = slice(th * SK, (th + 1) * SK)
        in_maps.append({
            "xqb": xfb[b],
            "xkv": np.ascontiguousarray(xf[b][:, ksl]),
            "xkvb": np.ascontiguousarray(xfb[b][:, ksl]),
            "wqtb": wqtb,
            "wmtb": wmtb,
            "wvt": wvt,
            "wvtb": wvtb,
            "sel": sel,
        })
    return in_maps


def kernel(x, Wq, Wm, Wv):
    global LAST_EXEC_TIME_NS, LAST_RESULTS
    import time as _time
    _t = _time.time()
    nc = _get_program()
    print(f"[kernel] program built+compiled in {_time.time()-_t:.1f}s", flush=True)
    in_maps = _prep_in_maps(x, Wq, Wm, Wv)
    print("[kernel] starting run_bass_kernel_spmd", flush=True)
    _t = _time.time()

    trace = os.environ.get("ATT_KERNEL_TRACE", "0") == "1"
    kwargs = {}
    if trace:
        kwargs["trace"] = True
        tc_env = os.environ.get("ATT_KERNEL_TRACE_CORES", "0")
        kwargs["trace_cores"] = [int(c) for c in tc_env.split(",")]
    res = bass_utils.run_bass_kernel_spmd(
        nc, in_maps, core_ids=list(range(8)), **kwargs
    )
    print(f"[kernel] run_bass_kernel_spmd done in {_time.time()-_t:.1f}s", flush=True)
    LAST_EXEC_TIME_NS = res.exec_time_ns
    LAST_RESULTS = res

    R = np.empty((B, C2, T, THW), np.float32)
    V = np.empty((B, C2, THW), np.float32)
    for core in range(8):
        b, th = core // 2, core % 2
        r = res.results[core]["r"]          # (THW, TK, C2)
        R[b, :, th * TK:(th + 1) * TK, :] = r.transpose(2, 1, 0)
        V[b, :, th * SK:(th + 1) * SK] = res.results[core]["v"]
    return R.reshape(B, C2, T, T, H, W), V.reshape(B, C2, T, H, W)


# revision 9
# speedup vs baseline: 1.1405x; 1.1405x over previous
"""Trainium2 Bass kernel for nn_AttendModule (sparse spatial attention).

Computation (per batch b):
    Q = (Wq/sqrt(C2)) @ x ; M = Wm @ x ; V = Wv @ x          (1x1x1 conv projections)
    A[q, t, s] = <Q[:, q], M[:, t*S+s]>                       (QK over channels)
    Att = softmax over s (spatial positions within each key frame t)
    R[c, t, q] = sum_s Att[q, t, s] * V[c, t*S+s]
    returns (R reshaped (B, C2, T, T, H, W), V (B, C2, T, H, W))

Sharding over 8 NeuronCores: core = (b, th) with b = core//2 batch and
th = core%2 a half of the key-frame axis (8 key frames each). Queries are
kept whole on every core; key frames are disjoint, so no collectives.

Per-core dataflow (all on one core, SPMD program identical across cores):
    - projections via TensorE (bf16 operands, fp32 PSUM accumulation;
      V projection in full fp32 since V is returned directly)
    - A^T tiles (keys x queries) = M_sub^T @ Q via TensorE fp32-accum
    - exp on ScalarE (PSUM -> bf16 SBUF)
    - fused AV pass: for each 128-query subtile, R[q, t, c] accumulates
      E_j^T @ V^T_j (N=128) and, reusing the same stationary operand,
      D[q, t] accumulates E_j^T @ sel_j (N=8) -> softmax denominators
      land in query-partition layout, so the final normalize is a single
      DVE tensor_mul with a broadcast access pattern fused with the
      PSUM->SBUF evacuation.
"""
import os
import sys

sys.path.insert(0, "/opt/trn_rl_repo")

import numpy as np
import ml_dtypes

import concourse.bass as bass
import concourse.bacc as bacc
import concourse.mybir as mybir
import concourse.tile as tile
from concourse import bass_utils

B, C, T, H, W = 4, 256, 16, 16, 16
C2 = 128
S = H * W            # 256 spatial positions per frame
THW = T * S          # 4096 flattened query positions
TK = T // 2          # 8 key frames per core
SK = TK * S          # 2048 key positions per core
NCH = THW // 512     # 8 query chunks of 512

F32 = mybir.dt.float32
F32R = mybir.dt.float32r
BF16 = mybir.dt.bfloat16
EXP = mybir.ActivationFunctionType.Exp

LAST_EXEC_TIME_NS = None
LAST_RESULTS = None

_cache = {}


def _build_program():
    nc = bacc.Bacc("TRN2", target_bir_lowering=False, debug=False)

    xqb_d = nc.dram_tensor("xqb", [C, THW], BF16, kind="ExternalInput")
    xkv_d = nc.dram_tensor("xkv", [C, SK], F32, kind="ExternalInput")
    xkvb_d = nc.dram_tensor("xkvb", [C, SK], BF16, kind="ExternalInput")
    wqtb_d = nc.dram_tensor("wqtb", [C, C2], BF16, kind="ExternalInput")
    wmtb_d = nc.dram_tensor("wmtb", [C, C2], BF16, kind="ExternalInput")
    wvt_d = nc.dram_tensor("wvt", [C, C2], F32, kind="ExternalInput")
    wvtb_d = nc.dram_tensor("wvtb", [C, C2], BF16, kind="ExternalInput")
    sel_d = nc.dram_tensor("sel", [128, 128], BF16, kind="ExternalInput")
    r_d = nc.dram_tensor("r", [THW, TK, C2], F32, kind="ExternalOutput")
    v_d = nc.dram_tensor("v", [C2, SK], F32, kind="ExternalOutput")

    with tile.TileContext(nc) as tc:
        with (
            tc.tile_pool(name="const", bufs=1) as cpool,
            tc.tile_pool(name="xp", bufs=1) as xpool,
            tc.tile_pool(name="big", bufs=1) as bpool,
            tc.tile_pool(name="ep", bufs=32) as epool,
            tc.tile_pool(name="rp", bufs=4) as rpool,
            tc.tile_pool(name="pa", bufs=3, space="PSUM") as pa_pool,
            tc.tile_pool(name="pr", bufs=2, space="PSUM") as pr_pool,
            tc.tile_pool(name="pd", bufs=1, space="PSUM") as pd_pool,
        ):
            # ---- load inputs to SBUF
            xqb = [xpool.tile([128, THW], BF16, tag=f"xqb{h}", name=f"xqb{h}") for h in range(2)]
            xkv = [xpool.tile([128, SK], F32, tag=f"xkv{h}", name=f"xkv{h}") for h in range(2)]
            xkvb = [xpool.tile([128, SK], BF16, tag=f"xkvb{h}", name=f"xkvb{h}") for h in range(2)]
            wq = [cpool.tile([128, C2], BF16, tag=f"wq{h}", name=f"wq{h}") for h in range(2)]
            wm = [cpool.tile([128, C2], BF16, tag=f"wm{h}", name=f"wm{h}") for h in range(2)]
            wv = [cpool.tile([128, C2], F32, tag=f"wv{h}", name=f"wv{h}") for h in range(2)]
            wvb = [cpool.tile([128, C2], BF16, tag=f"wvb{h}", name=f"wvb{h}") for h in range(2)]
            for h in range(2):
                hs = slice(h * 128, (h + 1) * 128)
                nc.sync.dma_start(wq[h][:], wqtb_d.ap()[hs, :])
                nc.sync.dma_start(wm[h][:], wmtb_d.ap()[hs, :])
                nc.sync.dma_start(wv[h][:], wvt_d.ap()[hs, :])
                nc.sync.dma_start(wvb[h][:], wvtb_d.ap()[hs, :])
            sel = cpool.tile([128, 128], BF16, tag="sel")
            nc.sync.dma_start(sel[:], sel_d.ap())
            for c in range(0, THW, 1024):
                cs = slice(c, c + 1024)
                for h in range(2):
                    hs = slice(h * 128, (h + 1) * 128)
                    nc.gpsimd.dma_start(xqb[h][:, cs], xqb_d.ap()[hs, cs])
                for h in range(2):
                    hs = slice(h * 128, (h + 1) * 128)
                    if c < SK:
                        nc.gpsimd.dma_start(xkvb[h][:, cs], xkvb_d.ap()[hs, cs])
            for c in range(0, SK, 1024):
                cs = slice(c, c + 1024)
                for h in range(2):
                    hs = slice(h * 128, (h + 1) * 128)
                    nc.gpsimd.dma_start(xkv[h][:, cs], xkv_d.ap()[hs, cs])

            Q_sb = bpool.tile([128, THW], BF16, tag="Q")
            M_sb = bpool.tile([128, SK], BF16, tag="M")
            V_sb = bpool.tile([128, SK], F32, tag="V")
            VT_sb = bpool.tile([128, SK], BF16, tag="VT")

            # ---- Q and M projections (feed QK immediately)
            for n in range(4):
                for w_, x_, dst in ((wq, xqb, Q_sb), (wm, xkvb, M_sb)):
                    ns = slice(n * 512, (n + 1) * 512)
                    p = pa_pool.tile([128, 512], F32, tag="pa", name="p")
                    nc.tensor.matmul(p[:], w_[0][:], x_[0][:, ns], start=True, stop=False)
                    nc.tensor.matmul(p[:], w_[1][:], x_[1][:, ns], start=False, stop=True)
                    nc.scalar.copy(dst[:, ns], p[:])
            for n in range(4, NCH):
                ns = slice(n * 512, (n + 1) * 512)
                p = pa_pool.tile([128, 512], F32, tag="pa", name="p")
                nc.tensor.matmul(p[:], wq[0][:], xqb[0][:, ns], start=True, stop=False)
                nc.tensor.matmul(p[:], wq[1][:], xqb[1][:, ns], start=False, stop=True)
                nc.scalar.copy(Q_sb[:, ns], p[:])

            def vt_proj(jlist):
                # V^T (key-partition layout) for the AV matmul, bf16
                for j in jlist:
                    js = slice(j * 128, (j + 1) * 128)
                    p = pd_pool.tile([128, 128], F32, tag="pd", name="p")
                    nc.tensor.matmul(p[:], xkvb[0][:, js], wvb[0][:], start=True, stop=False)
                    nc.tensor.matmul(p[:], xkvb[1][:, js], wvb[1][:], start=False, stop=True)
                    nc.scalar.copy(VT_sb[:, js], p[:])

            def v_proj(nlist):
                # full-fp32 V projection (returned output; only feeds the DMA)
                for n in nlist:
                    ns = slice(n * 512, (n + 1) * 512)
                    p = pa_pool.tile([128, 512], F32, tag="pa", name="p")
                    nc.tensor.matmul(p[:], wv[0][:], xkv[0][:, ns], start=True, stop=False)
                    nc.tensor.matmul(p[:], wv[1][:], xkv[1][:, ns], start=False, stop=True)
                    nc.vector.tensor_copy(V_sb[:, ns], p[:])
                    nc.gpsimd.dma_start(v_d.ap()[:, ns], V_sb[:, ns])

            # ---- attention, one 512-query chunk at a time
            for n in range(NCH):
                ns = slice(n * 512, (n + 1) * 512)
                E = []
                for j in range(16):
                    js = slice(j * 128, (j + 1) * 128)
                    p = pa_pool.tile([128, 512], F32, tag="pa")
                    nc.tensor.matmul(p[:], M_sb[:, js], Q_sb[:, ns], start=True, stop=True)
                    e = epool.tile([128, 512], BF16, tag="E")
                    nc.scalar.activation(e[:], p[:], EXP)
                    E.append(e)
                if n == 0:
                    vt_proj(range(16))
                elif n == 1:
                    v_proj(range(2))
                elif n == 2:
                    v_proj(range(2, 4))
                rsb = rpool.tile([128, 4, TK, C2], F32, tag="rsb")
                for u in range(4):
                    us = slice(u * 128, (u + 1) * 128)
                    pr = pr_pool.tile([128, TK, C2], F32, tag="pr")
                    pdt = pd_pool.tile([128, TK], F32, tag="pd")
                    for j in range(16):
                        t = j // 2
                        nc.tensor.matmul(
                            pr[:, t, :], E[j][:, us], VT_sb[:, j * 128:(j + 1) * 128],
                            start=(j % 8 == 0), stop=(j % 8 == 7),
                        )
                        nc.tensor.matmul(
                            pdt[:], E[j][:, us], sel[:, j * 8:(j + 1) * 8],
                            start=(j == 0), stop=(j == 15),
                        )
                    rdt = rpool.tile([128, TK], F32, tag="rdt")
                    if os.environ.get("ATT_SAFE_RECIP", "0") == "1":
                        nc.vector.reciprocal(out=rdt[:], in_=pdt[:])
                    else:
                        nc.vector.reciprocal_approx_fast(out=rdt[:], in_=pdt[:])
                    for t in range(TK):
                        nc.vector.tensor_scalar_mul(
                            rsb[:, u, t, :], pr[:, t, :], rdt[:, t:t + 1]
                        )
                    q0 = n * 512 + u * 128
                    nc.sync.dma_start(r_d.ap()[q0:q0 + 128, :, :], rsb[:, u])

    nc.compile()
    return nc


def _get_program():
    if "nc" not in _cache:
        _cache["nc"] = _build_program()
    return _cache["nc"]


def _prep_in_maps(x, Wq, Wm, Wv):
    x = np.asarray(x, np.float32)
    Wq = np.asarray(Wq, np.float32)
    Wm = np.asarray(Wm, np.float32)
    Wv = np.asarray(Wv, np.float32)

    inv = np.float32(1.0 / np.sqrt(np.float32(C2)))
    xf = np.ascontiguousarray(x.reshape(B, C, THW))
    xfb = xf.astype(ml_dtypes.bfloat16)
    wqtb = np.ascontiguousarray((Wq.T * inv).astype(ml_dtypes.bfloat16))
    wmtb = np.ascontiguousarray(Wm.T.astype(ml_dtypes.bfloat16))
    wvt = np.ascontiguousarray(Wv.T)
    wvtb = wvt.astype(ml_dtypes.bfloat16)
    sel = np.zeros((128, 128), ml_dtypes.bfloat16)
    for j in range(16):
        sel[:, 8 * j + j // 2] = 1.0

    in_maps = []
    for core in range(8):
        b, th = core // 2, core % 2
        ksl = slice(th * SK, (th + 1) * SK)
        in_maps.append({
            "xqb": xfb[b],
            "xkv": np.ascontiguousarray(xf[b][:, ksl]),
            "xkvb": np.ascontiguousarray(xfb[b][:, ksl]),
            "wqtb": wqtb,
            "wmtb": wmtb,
            "wvt": wvt,
            "wvtb": wvtb,
            "sel": sel,
        })
    return in_maps


def kernel(x, Wq, Wm, Wv):
    global LAST_EXEC_TIME_NS, LAST_RESULTS
    import time as _time
    _t = _time.time()
    nc = _get_program()
    print(f"[kernel] program built+compiled in {_time.time()-_t:.1f}s", flush=True)
    in_maps = _prep_in_maps(x, Wq, Wm, Wv)
    print("[kernel] starting run_bass_kernel_spmd", flush=True)
    _t = _time.time()

    trace = os.environ.get("ATT_KERNEL_TRACE", "0") == "1"
    kwargs = {}
    if trace:
        kwargs["trace"] = True
        tc_env = os.environ.get("ATT_KERNEL_TRACE_CORES", "0")
        kwargs["trace_cores"] = [int(c) for c in tc_env.split(",")]
    res = bass_utils.run_bass_kernel_spmd(
        nc, in_maps, core_ids=list(range(8)), **kwargs
    )
    print(f"[kernel] run_bass_kernel_spmd done in {_time.time()-_t:.1f}s", flush=True)
    LAST_EXEC_TIME_NS = res.exec_time_ns
    LAST_RESULTS = res

    R = np.empty((B, C2, T, THW), np.float32)
    V = np.empty((B, C2, THW), np.float32)
    for core in range(8):
        b, th = core // 2, core % 2
        r = res.results[core]["r"]          # (THW, TK, C2)
        R[b, :, th * TK:(th + 1) * TK, :] = r.transpose(2, 1, 0)
        V[b, :, th * SK:(th + 1) * SK] = res.results[core]["v"]
    return R.reshape(B, C2, T, T, H, W), V.reshape(B, C2, T, H, W)
